# revision 2
# baseline (speedup 1.0000x reference)
"""Trainium2 Bass kernel for nn_ColonyCBF (gnn_message_passing).

Computation (per row b of B=2^21):
    x_flat = concat(x_local[b], x_all[b, 1:7, :])            # 28 features
    h1 = relu(x_flat @ W1 + b1)                              # 64
    h2 = relu(h1 @ W2 + b2)                                  # 32
    out = 0.3 - softmax(|rw|) . x_local[b] + 0.1*(h2 @ W3 + b3)

Strategy: pure data-parallel over 8 NeuronCores.  The host packs the batch
into a transposed 4-row "strip" layout (feature-on-partition, bf16): strip r
(partitions 32r..32r+31, 28 used) holds batch quarter POS[r]; 4 batch rows
stream per PE column.  The device kernel is a 128-chunk stream (N=512 cols =
2048 rows per chunk) of:

  L1:   two row-tiled [K=64 -> M=128] matmuls (zero-padded block-diag W1),
        concurrent on disjoint row groups, into one 2-bank PSUM tile
  relu1: ONE [128,1024] op (alternating ACT/DVE), per-partition bias
  L2:   two col-tiled [128 -> 64] matmuls (block-diag W2) into one bank
  relu2: [128,512] op on the opposite engine
  risk/L3: M=100 matmuls with 4-chunk-packed outputs: chunk j%4=q writes
        rows 32q+i (strip i) of a shared PSUM bank via zero-padded wide
        weights; risk (vs x) starts the group, L3 (vs h2, software-pipelined
        one chunk behind) accumulates
  final: ONE [100,512] ACT per 4 chunks (bias 0.3+0.1*b3) -> stage
  store: one [100, 2048] DMA per 16 chunks -> y[100, QUARTER/4]
        (host picks the 16 meaningful rows)

Input DMA is prefetched in 2MB groups (16 chunks) one group ahead.
"""

import sys
import numpy as np
import ml_dtypes

sys.path.insert(0, "/opt/trn_rl_repo")

BF16 = ml_dtypes.bfloat16

B = 2_097_152
N_CORES = 8
BC = B // N_CORES            # rows per core
QUARTER = BC // 4            # columns of the packed layout
N = 512                      # columns (batch rows / 4) per chunk
N_CHUNKS = QUARTER // N      # 128
XGROUP = 16                  # chunks per input DMA (prefetched one ahead)

POS = (2, 0, 3, 1)           # strip r holds batch quarter POS[r]

_BUILD_CACHE = {}


def _build(repeat=1):
    key = repeat
    if key in _BUILD_CACHE:
        return _BUILD_CACHE[key]
    import concourse.mybir as mybir
    import concourse.tile as tile
    from concourse import bacc
    from concourse.alu_op_type import AluOpType
    from contextlib import ExitStack

    dt = mybir.dt
    AF = mybir.ActivationFunctionType

    nc = bacc.Bacc("TRN2", target_bir_lowering=False, debug=False,
                   num_devices=N_CORES)
    xt_d = nc.dram_tensor("xt", [128, QUARTER], dt.bfloat16,
                          kind="ExternalInput").ap()
    w1a_d = nc.dram_tensor("w1a", [128, 128], dt.bfloat16,
                           kind="ExternalInput").ap()
    w1b_d = nc.dram_tensor("w1b", [128, 128], dt.bfloat16,
                           kind="ExternalInput").ap()
    w2_d = nc.dram_tensor("w2", [128, 64], dt.bfloat16,
                          kind="ExternalInput").ap()
    w3w_d = nc.dram_tensor("w3w", [128, 400], dt.bfloat16,
                           kind="ExternalInput").ap()
    wrw_d = nc.dram_tensor("wrw", [128, 400], dt.bfloat16,
                           kind="ExternalInput").ap()
    b1r_d = nc.dram_tensor("b1r", [128, 1], dt.float32,
                           kind="ExternalInput").ap()
    b2r_d = nc.dram_tensor("b2r", [128, 1], dt.float32,
                           kind="ExternalInput").ap()
    bf_d = nc.dram_tensor("bfin", [128, 1], dt.float32,
                          kind="ExternalInput").ap()
    y_d = nc.dram_tensor("y", [100, QUARTER // 4], dt.float32,
                         kind="ExternalOutput").ap()

    with tile.TileContext(nc) as tc, ExitStack() as ctx:
        consts = ctx.enter_context(tc.tile_pool(name="consts", bufs=1))
        xpool = ctx.enter_context(tc.tile_pool(name="x", bufs=3))
        h1pool = ctx.enter_context(tc.tile_pool(name="h1", bufs=4))
        h2pool = ctx.enter_context(tc.tile_pool(name="h2", bufs=3))
        stpool = ctx.enter_context(tc.tile_pool(name="stage", bufs=2))
        psA = ctx.enter_context(tc.tile_pool(name="psA", bufs=2, space="PSUM"))
        psE = ctx.enter_context(tc.tile_pool(name="psE", bufs=2, space="PSUM"))
        psG = ctx.enter_context(tc.tile_pool(name="psG", bufs=2, space="PSUM"))

        def cl(dram, shape, dtype):
            t = consts.tile(shape, dtype, tag=dram.tensor.name)
            nc.sync.dma_start(out=t, in_=dram)
            return t

        s_w1a = cl(w1a_d, [128, 128], dt.bfloat16)
        s_w1b = cl(w1b_d, [128, 128], dt.bfloat16)
        s_w2 = cl(w2_d, [128, 64], dt.bfloat16)
        s_w3w = cl(w3w_d, [128, 400], dt.bfloat16)
        s_wrw = cl(wrw_d, [128, 400], dt.bfloat16)
        s_b1r = cl(b1r_d, [128, 1], dt.float32)
        s_b2r = cl(b2r_d, [128, 1], dt.float32)
        s_bf = cl(bf_d, [128, 1], dt.float32)

        N_XG = N_CHUNKS // XGROUP

        def body():
            state = {"xbigs": {}, "stages": {}, "pR": None}
            prevL3 = {"h2t": None, "j": -1, "pR": None}

            def prefetch(g):
                if g >= N_XG or g in state["xbigs"]:
                    return
                xb = xpool.tile([128, XGROUP * N], dt.bfloat16,
                                name="xbig", tag="xbig")
                nc.sync.dma_start(out=xb,
                                  in_=xt_d[:, g * XGROUP * N:(g + 1) * XGROUP * N])
                state["xbigs"][g] = xb

            def emit_L3():
                """L3 + (every 4 chunks) final/store for the pipelined
                previous chunk."""
                h2t, jp, pR = prevL3["h2t"], prevL3["j"], prevL3["pR"]
                if h2t is None:
                    return
                q = jp % 4
                nc.tensor.matmul(pR[0:100, :], s_w3w[:, 100 * q:100 * (q + 1)],
                                 h2t, start=False, stop=(q == 3),
                                 skip_group_check=True)
                prevL3["h2t"] = None
                if q == 3:
                    g4 = jp // 4
                    stage = state["stages"][g4 // 4]
                    nc.scalar.activation(
                        stage[0:100, (g4 % 4) * N:(g4 % 4 + 1) * N],
                        pR[0:100, :], AF.Identity, bias=s_bf[0:100], scale=1.0)
                    if g4 % 4 == 3:
                        s = g4 // 4
                        nc.sync.dma_start(
                            out=y_d[:, s * 4 * N:(s + 1) * 4 * N],
                            in_=stage[0:100, :])
                        del state["stages"][s]

            for j in range(N_CHUNKS):
                if j % 16 == 0:
                    state["stages"][j // 16] = stpool.tile(
                        [128, 4 * N], dt.float32, name="stage", tag="stage")
                g = j // XGROUP
                if j == 0:
                    prefetch(0)
                if j % XGROUP == 0:
                    prefetch(g + 1)
                xtile = state["xbigs"][g][:, (j % XGROUP) * N:(j % XGROUP + 1) * N]
                if j % XGROUP == XGROUP - 1:
                    state["xbigs"].pop(g - 1, None)
                q = j % 4
                pAB = psA.tile([128, 2 * N], dt.float32, name="pAB", tag="pAB")
                if q == 0:
                    state["pR"] = psG.tile([128, N], dt.float32, name="pRb",
                                           tag="pRb")
                pR = state["pR"]
                # L1 halves on disjoint row groups -> concurrent
                nc.tensor.matmul(pAB[:, 0:N], s_w1a[0:64, :], xtile[0:64, :],
                                 start=True, stop=True, tile_position=(0, 0))
                nc.tensor.matmul(pAB[:, N:2 * N], s_w1b[64:128, :],
                                 xtile[64:128, :], start=True, stop=True,
                                 tile_position=(64, 0))
                # risk into rows 32q+i of the group-shared bank; q==0 opens
                # the accumulation group (writes/clears all 100 rows)
                nc.tensor.matmul(pR[0:100, :], s_wrw[:, 100 * q:100 * (q + 1)],
                                 xtile, start=(q == 0), stop=False,
                                 skip_group_check=True)
                # pipelined tail of the previous chunk sits here so the
                # in-order PE never waits on this chunk's relu2
                emit_L3()
                h1ab = h1pool.tile([128, 2 * N], dt.bfloat16, tag="h1")
                if j % 2 == 0:
                    nc.scalar.activation(h1ab, pAB, AF.Relu, bias=s_b1r,
                                         scale=1.0)
                else:
                    nc.vector.tensor_scalar(out=h1ab, in0=pAB, scalar1=s_b1r,
                                            scalar2=0.0, op0=AluOpType.add,
                                            op1=AluOpType.max)
                pE = psE.tile([128, N], dt.float32)
                nc.tensor.matmul(pE[0:64, :], s_w2, h1ab[:, 0:N], start=True,
                                 stop=True, tile_position=(0, 0))
                nc.tensor.matmul(pE[64:128, :], s_w2, h1ab[:, N:2 * N],
                                 start=True, stop=True, tile_position=(0, 64))
                h2t = h2pool.tile([128, N], dt.bfloat16)
                if j % 2 == 0:
                    nc.vector.tensor_scalar(out=h2t, in0=pE, scalar1=s_b2r,
                                            scalar2=0.0, op0=AluOpType.add,
                                            op1=AluOpType.max)
                else:
                    nc.scalar.activation(h2t, pE, AF.Relu, bias=s_b2r,
                                         scale=1.0)
                prevL3.update(h2t=h2t, j=j, pR=pR)
            emit_L3()

        if repeat > 1:
            with tc.For_i(0, repeat, 1):
                body()
        else:
            body()

    nc.compile()
    _BUILD_CACHE[key] = nc
    return nc


def _prep_inputs(x_local, x_all, W1, b1, W2, b2, W3, b3, risk_weights):
    xf = np.empty((B, 28), np.float32)
    xf[:, :4] = x_local
    xf[:, 4:] = x_all[:, 1:7, :].reshape(B, 24)
    xb = xf.astype(BF16)
    X = xb.reshape(N_CORES, 4, QUARTER, 28)

    w1a = np.zeros((128, 128), BF16)
    w1a[0:28, 0:64] = W1
    w1a[32:60, 64:128] = W1
    w1b = np.zeros((128, 128), BF16)
    w1b[64:92, 0:64] = W1
    w1b[96:124, 64:128] = W1
    w2m = np.zeros((128, 64), BF16)
    w2m[0:64, 0:32] = W2
    w2m[64:128, 32:64] = W2
    a = np.abs(np.asarray(risk_weights, np.float32))
    e = np.exp(a - a.max())
    wsm = e / e.sum()
    wrw = np.zeros((128, 400), BF16)
    w3w = np.zeros((128, 400), BF16)
    w3c = 0.1 * np.asarray(W3, np.float32)[:, 0]
    for q in range(4):
        for i in range(4):
            wrw[32 * i:32 * i + 4, 100 * q + 32 * q + i] = -wsm
            w3w[32 * i:32 * i + 32, 100 * q + 32 * q + i] = w3c
    b1r = np.tile(np.asarray(b1, np.float32), 2).reshape(128, 1)
    b2r = np.tile(np.asarray(b2, np.float32), 4).reshape(128, 1)
    bfin = np.full((128, 1), 0.3 + 0.1 * float(b3[0]), np.float32)

    consts = dict(w1a=w1a, w1b=w1b, w2=w2m, w3w=w3w, wrw=wrw,
                  b1r=b1r, b2r=b2r, bfin=bfin)
    in_maps = []
    for c in range(N_CORES):
        xt = np.zeros((4, 32, QUARTER), BF16)
        # strip r holds batch quarter POS[r]
        xt[:, :28, :] = X[c][list(POS)].transpose(0, 2, 1)
        in_maps.append(dict(xt=xt.reshape(128, QUARTER), **consts))
    return in_maps


def run(in_maps, repeat=1):
    from concourse.bass_utils import run_bass_kernel_spmd
    nc = _build(repeat)
    return run_bass_kernel_spmd(nc, in_maps, core_ids=list(range(N_CORES)))


def kernel(x_local, x_all, W1, b1, W2, b2, W3, b3, risk_weights):
    x_local = np.asarray(x_local)
    x_all = np.asarray(x_all)
    in_maps = _prep_inputs(x_local, x_all, np.asarray(W1), np.asarray(b1),
                           np.asarray(W2), np.asarray(b2), np.asarray(W3),
                           np.asarray(b3), np.asarray(risk_weights))
    res = run(in_maps)
    out = np.empty(B, np.float32)
    idx = [32 * q + i for q in range(4) for i in range(4)]
    for c in range(N_CORES):
        y0 = np.asarray(res.results[c]["y"], np.float32)   # [100, QUARTER/4]
        arr = y0[idx].reshape(4, 4, N_CHUNKS // 4, N)      # [q, i, g4, n]
        arr = arr.transpose(1, 2, 0, 3).reshape(4, QUARTER)  # strip i, pos
        base = c * BC
        for i in range(4):
            qq = POS[i]
            out[base + qq * QUARTER: base + (qq + 1) * QUARTER] = arr[i]
    return out
